# revision 5
# baseline (speedup 1.0000x reference)
"""Trainium2 Bass kernel for nn_DepthCalibration.

Math (per batch b):
  s      = conv1d(pred*g, w, pad=1) + cb                     (smoothed depths)
  e[n,m] = -2*||ray_n - ray_m||^2                            (sigma=0.5 fixed)
  out[n] = clip(sum_m exp(e[n,m]) * s[m], 0.1, 100)

Strategy: one batch per NeuronCore (B=8, 8 cores, fully data parallel).

v2 (symmetric upper-triangle + folded weights):
  The Gaussian weight matrix W = exp(e) is symmetric, so only the upper
  triangle (incl. diagonal blocks) is materialized -- halving the ACT
  exp work, which is the hard per-core floor (1 elem/lane/cycle @1.2GHz
  => ~56us for N^2/2).  When all s[m] share one sign (true for the
  reference seed: conv taps sum negative), fold t=sigma*s>0 into the
  exponent as a 10th contraction row ln(t[m]):
      W''[n,m] = exp(e[n,m] + ln t[m]) = W[n,m] * t[m]
  Then:
    - row part  out_r[n] = sum_{m>=blk(n)} W''[n,m]   comes for free from
      the ACT instruction's accum_out (per-partition running sum), so the
      DVE never touches the N^2 data at all (baseline burned ~146us/core
      on a full-matrix DVE STT).
    - col part (strict lower triangle, via symmetry) is computed on the
      PE: for block (i,j), i<j, a matmul with the W'' block as the
      128x128 fp16 stationary and t_i [128,1] as moving accumulates
      P[m] += sum_n W''[n,m] t_i[n] = t[m] * sum_n W[n,m] t[n]
      into a persistent PSUM bank (start/stop accumulation over i);
      one O(N) divide by t[m] at the end recovers the column sums.
  Final: out = clip(sigma*(rowsum + colsum/t), 0.1, 100).

Exponent matmuls are rank-10 f32r (1 cyc/row @2.4GHz for >=256-wide
moving, ~28us/core for the upper triangle).  Engine budget per core:
ACT exp ~56us + overheads, PE ~55us (exponent + 496 colsum LDWEIGHTS),
DVE ~0; all overlapped.

Mixed-sign s falls back to the v1 full-matrix path (build_program_full).
"""

import sys
import os

sys.path.insert(0, "/opt/trn_rl_repo")

import numpy as np

from concourse import bass, mybir
from concourse import bacc
from concourse import tile
from concourse.bass_utils import run_bass_kernel_spmd

B, N = 8, 4096
NB = N // 128          # 32 row blocks of 128
MM = 512               # matmul moving free dim (one PSUM bank of fp32)
CH = 1536              # ACT/exp chunk = 3 PSUM banks
MAXCHUNKS = 3          # max chunks per strip (ceil(4096/1536))
MIN_DEPTH, MAX_DEPTH = 0.1, 100.0

F32 = mybir.dt.float32
F32R = mybir.dt.float32r
FP16 = mybir.dt.float16

KAUG = 10              # augmented contraction depth (incl. ln t row)
ALT = True             # alternate PE row groups to hide LDWEIGHTS


def build_program(gw0, gw1, gw2, cb, sign, w_dtype=FP16, repeat=1):
    """Build the single-core symmetric program (run SPMD on 8 cores).

    gw0/gw1/gw2/cb: conv taps and bias pre-multiplied by global_scale AND
    by `sign` (so the conv produces t = sign*s > 0).  The final result is
    multiplied by `sign` before clipping.
    repeat>1 wraps the body in a hardware loop (for timing measurement).
    """
    nc = bacc.Bacc(
        "TRN2",
        target_bir_lowering=False,
        debug=False,
        enable_asserts=False,
        num_devices=8,
    )

    pred_pad = nc.dram_tensor("pred_pad", (N + 2,), F32, kind="ExternalInput").ap()
    rayT = nc.dram_tensor("rayT", (3, N), F32, kind="ExternalInput").ap()
    out = nc.dram_tensor("out", (N,), F32, kind="ExternalOutput").ap()
    ln_dram = nc.dram_tensor("ln_scratch", (N,), F32, kind="Internal").ap()

    AF = mybir.ActivationFunctionType
    OP = mybir.AluOpType

    from contextlib import ExitStack

    ngrp = 2 if ALT else 1

    with tile.TileContext(nc) as tc, ExitStack() as stk:
        if repeat > 1:
            ET = mybir.EngineType
            stk.enter_context(
                tc.For_i(
                    0,
                    repeat,
                    1,
                    hint_engines=(ET.PE, ET.DVE, ET.Activation, ET.SP, ET.Pool),
                )
            )
        with (
            tc.tile_pool(name="const", bufs=1) as cpool,
            tc.tile_pool(name="w", bufs=3) as wpool,
            tc.tile_pool(name="psum", bufs=2, space="PSUM") as ppool,
            tc.tile_pool(name="cs", bufs=1, space="PSUM") as cspool,
        ):
            # ---------------- aug matrices A (stationary) and B (moving) ----
            # e + ln t = matmul(A[:,n], B[:,m]) with
            #   A = [x, y, z, x^2, y^2, z^2, -2, -2, -2, 1]
            #   B = [4x', 4y', 4z', -2, -2, -2, x'^2, y'^2, z'^2, ln t']
            # duplicated at base partition 32 so consecutive row blocks use
            # different PE row groups (LDWEIGHTS overlaps in-flight matmuls)
            A = cpool.tile([32 * (ngrp - 1) + KAUG, N], F32R)
            Bm = cpool.tile([32 * (ngrp - 1) + KAUG, N], F32R)
            R = cpool.tile([3, N], F32)      # raw rays (x,y,z rows)
            sqm = cpool.tile([3, N], F32R)   # x^2 ...
            r4 = cpool.tile([3, N], F32R)    # 4x ...
            ones3 = nc.inline_tensor(np.ones((3, N), np.float32), "ones3").ap()
            m2s3 = nc.inline_tensor(np.full((3, N), -2.0, np.float32), "m2s3").ap()

            nc.sync.dma_start(R[:], rayT[:, :])
            nc.scalar.activation(A[0:3, :], R[:], AF.Identity)
            nc.scalar.activation(sqm[:], R[:], AF.Square)
            nc.vector.tensor_scalar_mul(r4[:], R[:], 4.0)
            nc.sync.dma_start(A[3:6, :], sqm[:])
            nc.sync.dma_start(A[6:9, :], m2s3.bitcast(F32R))
            nc.sync.dma_start(A[9:10, :], ones3[0:1, :].bitcast(F32R))
            nc.sync.dma_start(Bm[0:3, :], r4[:])
            nc.sync.dma_start(Bm[3:6, :], m2s3.bitcast(F32R))
            nc.sync.dma_start(Bm[6:9, :], sqm[:])

            # ---------------- smoothed depths t = sign*s (vertical) ---------
            # V*[p, c] = pred_pad[off + p + 128c];  t[i] for i = p + 128c
            def vload(off):
                t = cpool.tile([128, NB], F32, tag=f"v{off}")
                src = pred_pad[off : off + N].rearrange("(c p) -> p c", p=128)
                nc.sync.dma_start(t[:], src)
                return t

            sv = cpool.tile([128, NB], F32)
            vl, vc, vr = vload(0), vload(1), vload(2)
            nc.vector.tensor_scalar_mul(sv[:], vl[:], gw0)
            nc.vector.scalar_tensor_tensor(
                sv[:], vc[:], gw1, sv[:], OP.mult, OP.add
            )
            nc.vector.scalar_tensor_tensor(
                sv[:], vr[:], gw2, sv[:], OP.mult, OP.add
            )
            nc.vector.tensor_scalar_add(sv[:], sv[:], cb)
            sv_c = cpool.tile([128, NB], w_dtype)   # t as fp16 (colsum moving)
            nc.vector.tensor_copy(sv_c[:], sv[:])
            # 1/t with one Newton step: recip = r0*(2 - t*r0)
            recip = cpool.tile([128, NB], F32)
            r0 = cpool.tile([128, NB], F32)
            nc.vector.reciprocal(r0[:], sv[:])
            tr = cpool.tile([128, NB], F32)
            nc.vector.tensor_mul(tr[:], sv[:], r0[:])
            nc.vector.tensor_scalar(tr[:], tr[:], -1.0, 2.0, OP.mult, OP.add)
            nc.vector.tensor_mul(recip[:], r0[:], tr[:])
            # ln t, refined against the Exp table:
            #   L0 = Ln(t); u = exp(-L0); r = t*u - 1; ln t ~= L0 + r - r^2/2
            # (the raw Ln table's absolute error would otherwise scale every
            # folded weight by e^err)
            ln_sv = cpool.tile([128, NB], F32)
            l0 = cpool.tile([128, NB], F32)
            nc.scalar.activation(l0[:], sv[:], AF.Ln)
            uex = cpool.tile([128, NB], F32)
            nc.scalar.activation(uex[:], l0[:], AF.Exp, scale=-1.0)
            rr = cpool.tile([128, NB], F32)
            nc.vector.tensor_mul(rr[:], sv[:], uex[:])
            nc.vector.tensor_scalar_add(rr[:], rr[:], -1.0)
            rr2 = cpool.tile([128, NB], F32)
            nc.vector.tensor_mul(rr2[:], rr[:], rr[:])
            nc.vector.scalar_tensor_tensor(
                rr2[:], rr2[:], -0.5, rr[:], OP.mult, OP.add
            )
            nc.vector.tensor_add(ln_sv[:], l0[:], rr2[:])
            # vertical -> linear DRAM -> [1, N] row of B
            nc.sync.dma_start(ln_dram.rearrange("(c p) -> p c", p=128), ln_sv[:])
            nc.sync.dma_start(
                Bm[9:10, :],
                ln_dram.rearrange("(o n) -> o n", o=1).bitcast(F32R),
            )
            for g in range(1, ngrp):
                nc.sync.dma_start(A[32 * g : 32 * g + KAUG, :], A[0:KAUG, :])
                nc.sync.dma_start(Bm[32 * g : 32 * g + KAUG, :], Bm[0:KAUG, :])

            # ---------------- accumulators ----------------------------------
            cs = cspool.tile([128, NB], F32)        # colsum PSUM (1 bank)
            rsp = cpool.tile([128, NB * MAXCHUNKS], F32)  # rowsum partials
            nc.vector.memset(rsp[:], 0.0)
            nc.vector.memset(cs[:, 0:1], 0.0)       # block 0 has no col part

            # ---------------- main loop: upper-tri + diag strips ------------
            for i in range(NB):
                g = 32 * (i % ngrp)
                lhsT = A[g : g + KAUG, i * 128 : (i + 1) * 128]
                for k, m0 in enumerate(range(128 * i, N, CH)):
                    ch = min(CH, N - m0)
                    pt = ppool.tile([128, CH], F32, tag="ps")
                    off = 0
                    while off < ch:
                        mm = min(MM, ch - off)
                        nc.tensor.matmul(
                            pt[:, off : off + mm],
                            lhsT,
                            Bm[g : g + KAUG, m0 + off : m0 + off + mm],
                        )
                        off += mm
                    wt = wpool.tile([128, CH], w_dtype, tag="w")
                    nc.scalar.activation(
                        wt[:, :ch],
                        pt[:, :ch],
                        AF.Exp,
                        accum_out=rsp[:, i * MAXCHUNKS + k : i * MAXCHUNKS + k + 1],
                    )
                    # transpose (col-sum) contributions via PE accumulation.
                    # PSUM `has_written` semantics: start=True clears the
                    # *whole bank's* bits, so issue it only on the globally
                    # first cs matmul; afterwards the per-element bit gives
                    # overwrite-on-first-touch / accumulate-after for every
                    # column naturally.
                    j0 = m0 // 128
                    for jj in range(ch // 128):
                        j = j0 + jj
                        if j == i:
                            continue
                        nc.tensor.matmul(
                            cs[:, j : j + 1],
                            wt[:, jj * 128 : (jj + 1) * 128],
                            sv_c[:, i : i + 1],
                            start=(i == 0 and j == 1),
                            stop=(i == NB - 2 and j == NB - 1),
                            skip_group_check=True,
                        )

            # ---------------- combine + clip + store ------------------------
            acc = cpool.tile([128, NB], F32)
            nc.vector.tensor_add(acc[:], rsp[:, 0::MAXCHUNKS], rsp[:, 1::MAXCHUNKS])
            nc.vector.tensor_add(acc[:], acc[:], rsp[:, 2::MAXCHUNKS])
            fix = cpool.tile([128, NB], F32)
            nc.vector.tensor_mul(fix[:], cs[:, 0:NB], recip[:])
            nc.vector.tensor_add(acc[:], acc[:], fix[:])
            res = cpool.tile([128, NB], F32)
            nc.vector.tensor_scalar(
                res[:], acc[:], float(sign), MIN_DEPTH, OP.mult, OP.max
            )
            nc.vector.tensor_scalar_min(res[:], res[:], MAX_DEPTH)
            nc.sync.dma_start(out.rearrange("(i p) -> p i", p=128), res[:])

    nc.compile()
    return nc


# ---------------- v1 full-matrix fallback (mixed-sign s) --------------------

def build_program_full(gw0, gw1, gw2, cb, w_dtype=FP16, repeat=1):
    """v1 path: full N^2 matrix, DVE STT weighted row sums (no symmetry)."""
    nc = bacc.Bacc(
        "TRN2",
        target_bir_lowering=False,
        debug=False,
        enable_asserts=False,
        num_devices=8,
    )
    CHUNK = 2048
    NCHUNK = N // CHUNK
    K9 = 9

    pred_pad = nc.dram_tensor("pred_pad", (N + 2,), F32, kind="ExternalInput").ap()
    rayT = nc.dram_tensor("rayT", (3, N), F32, kind="ExternalInput").ap()
    out = nc.dram_tensor("out", (N,), F32, kind="ExternalOutput").ap()
    s_dram = nc.dram_tensor("s_scratch", (N,), w_dtype, kind="Internal").ap()

    AF = mybir.ActivationFunctionType
    OP = mybir.AluOpType

    from contextlib import ExitStack

    ngrp = 2

    with tile.TileContext(nc) as tc, ExitStack() as stk:
        if repeat > 1:
            ET = mybir.EngineType
            stk.enter_context(
                tc.For_i(
                    0, repeat, 1,
                    hint_engines=(ET.PE, ET.DVE, ET.Activation, ET.SP, ET.Pool),
                )
            )
        with (
            tc.tile_pool(name="const", bufs=1) as cpool,
            tc.tile_pool(name="w", bufs=3) as wpool,
            tc.tile_pool(name="ttr", bufs=2) as tpool,
            tc.tile_pool(name="psum", bufs=2, space="PSUM") as ppool,
        ):
            A = cpool.tile([32 * (ngrp - 1) + K9, N], F32R)
            Bmv = cpool.tile([32 * (ngrp - 1) + K9, N], F32R)
            R = cpool.tile([3, N], F32)
            sqm = cpool.tile([3, N], F32R)
            r4 = cpool.tile([3, N], F32R)
            ones3 = nc.inline_tensor(np.ones((3, N), np.float32), "ones3").ap()
            m2s3 = nc.inline_tensor(np.full((3, N), -2.0, np.float32), "m2s3").ap()

            nc.sync.dma_start(R[:], rayT[:, :])
            nc.scalar.activation(A[0:3, :], R[:], AF.Identity)
            nc.scalar.activation(sqm[:], R[:], AF.Square)
            nc.vector.tensor_scalar_mul(r4[:], R[:], 4.0)
            nc.sync.dma_start(A[3:6, :], sqm[:])
            nc.sync.dma_start(A[6:9, :], m2s3.bitcast(F32R))
            nc.sync.dma_start(Bmv[0:3, :], r4[:])
            nc.sync.dma_start(Bmv[6:9, :], sqm[:])
            nc.sync.dma_start(Bmv[3:6, :], m2s3.bitcast(F32R))
            for g in range(1, ngrp):
                nc.sync.dma_start(A[32 * g : 32 * g + K9, :], A[0:K9, :])
                nc.sync.dma_start(Bmv[32 * g : 32 * g + K9, :], Bmv[0:K9, :])

            def vload(off):
                t = cpool.tile([128, NB], F32, tag=f"v{off}")
                src = pred_pad[off : off + N].rearrange("(c p) -> p c", p=128)
                nc.sync.dma_start(t[:], src)
                return t

            sv = cpool.tile([128, NB], F32)
            vl, vc, vr = vload(0), vload(1), vload(2)
            nc.vector.tensor_scalar_mul(sv[:], vl[:], gw0)
            nc.vector.scalar_tensor_tensor(sv[:], vc[:], gw1, sv[:], OP.mult, OP.add)
            nc.vector.scalar_tensor_tensor(sv[:], vr[:], gw2, sv[:], OP.mult, OP.add)
            nc.vector.tensor_scalar_add(sv[:], sv[:], cb)
            sv_c = cpool.tile([128, NB], w_dtype)
            nc.vector.tensor_copy(sv_c[:], sv[:])
            nc.sync.dma_start(s_dram.rearrange("(c p) -> p c", p=128), sv_c[:])
            s_bc = cpool.tile([128, N], w_dtype)
            for q in range(4):
                sl = slice(q * (N // 4), (q + 1) * (N // 4))
                nc.sync.dma_start(
                    s_bc[:, sl],
                    s_dram[sl].rearrange("(o n) -> o n", o=1).broadcast_to(
                        (128, N // 4)
                    ),
                )

            acc = cpool.tile([128, NB], F32)
            for i in range(NB):
                g = 32 * (i % ngrp)
                lhsT = A[g : g + K9, i * 128 : (i + 1) * 128]
                wt = wpool.tile([128, N], w_dtype, tag="w")
                for c in range(NCHUNK):
                    m0 = c * CHUNK
                    pt = ppool.tile([128, CHUNK], F32, tag="ps")
                    for j in range(CHUNK // MM):
                        nc.tensor.matmul(
                            pt[:, j * MM : (j + 1) * MM],
                            lhsT,
                            Bmv[g : g + K9, m0 + j * MM : m0 + (j + 1) * MM],
                        )
                    nc.scalar.activation(wt[:, m0 : m0 + CHUNK], pt[:], AF.Exp)
                sc = tpool.tile([128, N], w_dtype, tag="sc")
                nc.vector.scalar_tensor_tensor(
                    sc[:], wt[:], 0.0, s_bc[:], OP.bypass, OP.mult,
                    accum_out=acc[:, i : i + 1],
                )

            res = cpool.tile([128, NB], F32)
            nc.vector.tensor_scalar(
                res[:], acc[:], MIN_DEPTH, MAX_DEPTH, OP.max, OP.min
            )
            nc.sync.dma_start(out.rearrange("(i p) -> p i", p=128), res[:])

    nc.compile()
    return nc


_cache = {}


def _host_s(pred_depth, conv_w, conv_b, global_scale):
    g = float(np.asarray(global_scale).reshape(-1)[0])
    w = np.asarray(conv_w, np.float32).reshape(-1)
    cb = float(np.asarray(conv_b).reshape(-1)[0])
    pp = np.pad(np.asarray(pred_depth, np.float32) * g, ((0, 0), (1, 1)))
    s = w[0] * pp[:, :-2] + w[1] * pp[:, 1:-1] + w[2] * pp[:, 2:] + cb
    return s, (float(w[0] * g), float(w[1] * g), float(w[2] * g)), cb


def get_program_for(pred_depth, conv_w, conv_b, global_scale, repeat=1):
    """Select + build (cached) the program variant for these inputs."""
    s, (gw0, gw1, gw2), cb = _host_s(pred_depth, conv_w, conv_b, global_scale)
    if s.min() > 0:
        sign = 1.0
    elif s.max() < 0:
        sign = -1.0
    else:
        sign = 0.0  # mixed: fall back to full-matrix path
    if sign != 0.0:
        key = ("sym", gw0, gw1, gw2, cb, sign, repeat)
        if key not in _cache:
            _cache[key] = build_program(
                sign * gw0, sign * gw1, sign * gw2, sign * cb, sign, repeat=repeat
            )
    else:
        key = ("full", gw0, gw1, gw2, cb, repeat)
        if key not in _cache:
            _cache[key] = build_program_full(gw0, gw1, gw2, cb, repeat=repeat)
    return _cache[key]


def kernel(pred_depth, ray_3d, conv_w, conv_b, global_scale, repeat=1):
    pred_depth = np.asarray(pred_depth, np.float32)
    ray_3d = np.asarray(ray_3d, np.float32)

    nc = get_program_for(pred_depth, conv_w, conv_b, global_scale, repeat=repeat)

    in_maps = []
    for b in range(B):
        pp = np.zeros(N + 2, np.float32)
        pp[1 : N + 1] = pred_depth[b]
        in_maps.append(
            {
                "pred_pad": pp,
                "rayT": np.ascontiguousarray(ray_3d[b].T),
            }
        )
    res = _run_with_retry(nc, in_maps)
    out = np.stack([res.results[b]["out"] for b in range(B)]).astype(np.float32)
    return out


def _run_with_retry(nc, in_maps, tries=3):
    # The shared axon device occasionally reports a transient
    # NRT_EXEC_UNIT_UNRECOVERABLE after a prior process crashed; it
    # recovers within ~20s. Retry rather than failing the whole call.
    import time as _time

    for attempt in range(tries):
        try:
            return run_bass_kernel_spmd(nc, in_maps, core_ids=list(range(B)))
        except Exception:
            if attempt == tries - 1:
                raise
            _time.sleep(25)
